# revision 30
# baseline (speedup 1.0000x reference)
"""Trainium2 Bass kernel for nn_DirectionalScan (2D directional diagonal-SSM + projection).

Math: for each of two directions (scan over h, scan over w),
    y[t] = sum_n Cm*Bm * sum_{u<=t} A^(t-u) x[u]  + D_skip*x[t]
then out = (y_h + y_v) @ Wp.T + b_proj.

Device decomposition: chunked SSM with chunk Q=16, all heavy work on the PE:
  - intra-chunk causal Toeplitz (kernel K[d,tau]=sum_n CB*A^tau, + D on the diagonal)
  - chunk-boundary states via a per-chunk increment matmul + a batched 3-step recurrence
  - inter-chunk contribution via a CB*A^(i+1) matmul accumulated into the same PSUM
  - fused output projection with Wp.T

Layout strategy (v2): the host pre-packs x into the PE-transposed
(j16,e8)-partition layout and all weights into SBUF-native [128, X] images, so
every load DMA is fully contiguous and the on-device transpose/permute phase of
v1 disappears.  Outputs are written in fp16 in the SBUF-native token-chunk
layout through the two hardware DGE queues; the host unpacks and scatter-adds.

Sharding: 8 cores; core k handles batch b=k//2 and half=k%2:
  vertical  (scan over w): sequences (b, h in [32*half, 32*half+32))
  horizontal(scan over h): sequences (b, w in [32*half, 32*half+32))
Each core projects its two partial y tensors separately (projection is linear);
the host scatter-adds the two 2048-token contributions into the full output.
"""
import os
from contextlib import ExitStack

import numpy as np

import concourse.bass as bass
import concourse.bacc as bacc
import concourse.tile as tile
from concourse import mybir
from concourse.bass_utils import run_bass_kernel_spmd
from concourse.masks import make_identity

F32 = mybir.dt.float32
F16 = mybir.dt.float16
NP_CDT = np.float16
B, H, W, D, N = 4, 64, 64, 512, 8
L, Q, C, SEQ = 64, 16, 4, 32   # seq len, chunk size, n chunks, seqs/core/direction
NOCT = 64                      # octets of 8 channels
NG = 32                        # 2-octet groups


# ----------------------------------------------------------------------------
# host-side packing
# ----------------------------------------------------------------------------

def _precompute_weights(A, Bm, Cm, D_skip, Wp):
    A64, B64, C64 = A.astype(np.float64), Bm.astype(np.float64), Cm.astype(np.float64)
    CB = C64 * B64                                   # [D, N]
    Apow = np.stack([A64 ** t for t in range(Q + 1)])  # [Q+1, D, N]
    Kconv = np.einsum("dn,tdn->dt", CB, Apow)        # [D, Q+1]
    T = np.zeros((D, Q, Q))
    for i in range(Q):
        for j in range(i + 1):
            T[:, i, j] = Kconv[:, i - j]
    T += np.eye(Q)[None] * D_skip.astype(np.float64)[:, None, None]

    # K-rows ordered (j16, d8): row = j*8 + d8
    W_T = np.zeros((128, NOCT, 128))
    W_P = np.zeros((128, NOCT, 64))
    for o in range(NOCT):
        for d8 in range(8):
            d = o * 8 + d8
            for j in range(Q):
                W_T[j * 8 + d8, o, d8::8] = T[d, :, j]
                W_P[j * 8 + d8, o, d8 * 8:d8 * 8 + 8] = Apow[Q - 1 - j, d]
    W_CBA = np.zeros((128, NG, 256))
    for g in range(NG):
        for o2 in range(2):
            for d8 in range(8):
                d = g * 16 + o2 * 8 + d8
                for n in range(N):
                    row = o2 * 64 + d8 * 8 + n
                    W_CBA[row, g, o2 * 128 + d8:o2 * 128 + 128:8] = (
                        CB[d, n] * Apow[1:Q + 1, d, n]
                    )
    A16 = np.zeros((128, NG))
    for g in range(NG):
        for o2 in range(2):
            for d8 in range(8):
                d = g * 16 + o2 * 8 + d8
                A16[o2 * 64 + d8 * 8:o2 * 64 + d8 * 8 + 8, g] = Apow[Q, d]
    A16 = np.repeat(A16, SEQ, axis=1)  # [128, (g32, s32)]
    # WPT[p, dc*512+dout] = Wp[dout, dc*128+p]
    WPT = np.ascontiguousarray(
        Wp.astype(np.float64).T.reshape(4, 128, D).transpose(1, 0, 2).reshape(128, 4 * D))
    return (W_T.reshape(128, NOCT * 128).astype(NP_CDT),
            W_P.reshape(128, NOCT * 64).astype(NP_CDT),
            W_CBA.reshape(128, NG * 256).astype(NP_CDT),
            A16.astype(NP_CDT), WPT.astype(NP_CDT))


def _pack_xt(x_dir):
    """x_dir [32 seq, 64 pos, 512 d] -> XT [128=(j16,e8), 64 oct * 128=(c4,s32)]."""
    v = x_dir.reshape(SEQ, C, Q, NOCT, 8)            # s c j o e
    v = v.transpose(2, 4, 3, 1, 0)                   # j e o c s
    return np.ascontiguousarray(v.reshape(128, NOCT * 128), dtype=NP_CDT)


# ----------------------------------------------------------------------------
# device program
# ----------------------------------------------------------------------------

def _phase_G(tc, pools, consts, xt, tag, q):
    nc = tc.nc
    (xt_pool, g_pool, s_pool, y_pool, yt_pool, out_pool,
     psA, psyw, psG, psout) = pools[:10]
    w_t_sb, w_p_sb, w_cba_sb, a16_sb, wpt_sb, ident = consts[:6]
    g_all = pools[-1][tag]["g_all"]
    ps_g = psG.tile([128, 512], F32, tag="ps_g")
    for k in range(8):
        o = q * 8 + k
        half = (o % 2) * 64
        col = (k // 2) * 128
        nc.tensor.matmul(
            ps_g[half:half + 64, col:col + 128],
            w_p_sb[:, o * 64:o * 64 + 64], xt[:, o * 128:(o + 1) * 128],
            start=True, stop=True, skip_group_check=True,
            tile_position=(0, half))
    if q % 2 == 0:
        nc.scalar.copy(g_all[:, q * 512:(q + 1) * 512], ps_g[:])
    else:
        nc.vector.tensor_copy(g_all[:, q * 512:(q + 1) * 512], ps_g[:])


def _recurrence(tc, pools, consts, tag):
    nc = tc.nc
    a16_sb = consts[3]
    st = pools[-1][tag]
    g_all, s_all = st["g_all"], st["s_all"]
    sv = s_all[:].rearrange("p (g c s) -> p g c s", g=NG, c=C, s=SEQ)
    gv = g_all[:].rearrange("p (g c s) -> p g c s", g=NG, c=C, s=SEQ)
    av = a16_sb.rearrange("p (g s) -> p g s", g=NG)
    nc.gpsimd.memset(sv[:, :, 0, :], 0.0)
    nc.vector.tensor_copy(sv[:, :, 1, :], gv[:, :, 0, :])
    for cc in (2, 3):
        nc.vector.tensor_mul(sv[:, :, cc, :], sv[:, :, cc - 1, :], av)
        nc.vector.tensor_add(sv[:, :, cc, :], sv[:, :, cc, :], gv[:, :, cc - 1, :])


def _phase_B(tc, pools, consts, xt, tag, og):
    nc = tc.nc
    (xt_pool, g_pool, s_pool, y_pool, yt_pool, out_pool,
     psA, psyw, psG, psout) = pools[:10]
    w_t_sb, w_p_sb, w_cba_sb, a16_sb, wpt_sb, ident = consts[:6]
    st = pools[-1][tag]
    s_all, y_sb = st["s_all"], st["y_sb"]
    ps_yw = psyw.tile([128, 512], F32, tag="ps_yw")
    for oo in range(4):
        o = og * 4 + oo
        nc.tensor.matmul(ps_yw[:, oo * 128:(oo + 1) * 128],
                         xt[:, o * 128:(o + 1) * 128],
                         w_t_sb[:, o * 128:(o + 1) * 128],
                         start=(oo == 0), stop=False, skip_group_check=True)
    for gg in range(2):
        g = og * 2 + gg
        nc.tensor.matmul(ps_yw[:, gg * 256:(gg + 1) * 256],
                         s_all[:, g * 128:(g + 1) * 128],
                         w_cba_sb[:, g * 256:(g + 1) * 256],
                         start=False, stop=(gg == 1), skip_group_check=True)
    y_dst = y_sb[:].rearrange("p (i og oo e) -> p og i oo e",
                              i=Q, og=16, oo=4, e=8)[:, og]
    ps_src = ps_yw[:].rearrange("p (oo i e) -> p i oo e", oo=4, i=Q, e=8)
    if og % 2 == 0:
        nc.vector.tensor_copy(y_dst, ps_src)
    else:
        nc.scalar.copy(y_dst, ps_src)


def _phase_proj(tc, pools, consts, z_dram, tag):
    """Software-pipelined projection: transpose-group(ph+1) is emitted before
    matmul-group(ph) so the PE has independent work during each yt copy."""
    nc = tc.nc
    (xt_pool, g_pool, s_pool, y_pool, yt_pool, out_pool,
     psA, psyw, psG, psout) = pools[:10]
    w_t_sb, w_p_sb, w_cba_sb, a16_sb, wpt_sb, ident = consts[:6]
    y_sb = pools[-1][tag]["y_sb"]
    yts, outs = {}, {}

    def emit_T(ph):
        ps_yt = psA.tile([128, 1024], F16, tag="ps_t")
        for ii2 in range(2):
            i = ph * 2 + ii2
            for dc in range(4):
                nc.tensor.transpose(
                    ps_yt[:, ii2 * 512 + dc * 128:ii2 * 512 + (dc + 1) * 128],
                    y_sb[:, i * 512 + dc * 128:i * 512 + (dc + 1) * 128], ident)
        yt = yt_pool.tile([128, 1024], F16, tag="yt")
        nc.vector.tensor_copy(yt[:], ps_yt[:])
        yts[ph] = yt

    def emit_MM(ph):
        iq, phq = ph // 2, ph % 2
        if phq == 0:
            outs[iq] = out_pool.tile([128, 4 * 512], F16, tag="osb",
                                     name=f"osb_{tag}_{iq}")
        out_sb = outs[iq]
        yt = yts.pop(ph)
        for ii2 in range(2):
            ii = phq * 2 + ii2
            ps_o = psout.tile([128, 512], F32, tag="ps_o")
            for dc in range(4):
                nc.tensor.matmul(ps_o[:], yt[:, ii2 * 512 + dc * 128:
                                              ii2 * 512 + (dc + 1) * 128],
                                 wpt_sb[:, dc * 512:(dc + 1) * 512],
                                 start=(dc == 0), stop=(dc == 3))
            if ii % 2 == 0:
                nc.scalar.copy(out_sb[:, ii * 512:(ii + 1) * 512], ps_o[:])
            else:
                nc.vector.tensor_copy(out_sb[:, ii * 512:(ii + 1) * 512], ps_o[:])
        nc.sync.dma_start(
            z_dram[:, iq * 2048 + phq * 1024:iq * 2048 + (phq + 1) * 1024],
            out_sb[:, phq * 1024:(phq + 1) * 1024])

    for ph in range(8):
        emit_T(ph)
        if ph > 0:
            emit_MM(ph - 1)
    emit_MM(7)


def _kernel_body(ctx, tc, aps):
    nc = tc.nc
    const_pool = ctx.enter_context(tc.tile_pool(name="consts", bufs=1))
    xt_pool = ctx.enter_context(tc.tile_pool(name="xt", bufs=2))
    g_pool = ctx.enter_context(tc.tile_pool(name="g", bufs=2))
    s_pool = ctx.enter_context(tc.tile_pool(name="s", bufs=2))
    y_pool = ctx.enter_context(tc.tile_pool(name="y", bufs=2))
    yt_pool = ctx.enter_context(tc.tile_pool(name="yt", bufs=3))
    out_pool = ctx.enter_context(tc.tile_pool(name="osb", bufs=3))
    psA = ctx.enter_context(tc.tile_pool(name="psA", bufs=2, space="PSUM"))
    psyw = ctx.enter_context(tc.tile_pool(name="psyw", bufs=2, space="PSUM"))
    psG = ctx.enter_context(tc.tile_pool(name="psG", bufs=2, space="PSUM"))
    psout = ctx.enter_context(tc.tile_pool(name="psout", bufs=2, space="PSUM"))
    pools = (xt_pool, g_pool, s_pool, y_pool, yt_pool, out_pool,
             psA, psyw, psG, psout)

    w_p_sb = const_pool.tile([128, NOCT * 64], F16, name="w_p_sb")
    a16_sb = const_pool.tile([128, NG * SEQ], F16, name="a16_sb")
    w_t_sb = const_pool.tile([128, NOCT * 128], F16, name="w_t_sb")
    w_cba_sb = const_pool.tile([128, NG * 256], F16, name="w_cba_sb")
    wpt_sb = const_pool.tile([128, 4 * 512], F16, name="wpt_sb")
    ident = const_pool.tile([128, 128], F16, name="ident")
    make_identity(nc, ident[:])
    # sync queue: xt_v, a16, xt_h, wpt (z writes follow later in program order)
    # scalar queue: w_p, w_t, w_cba
    xt_v = xt_pool.tile([128, NOCT * 128], F16, tag="xt", name="xt_v")
    xt_h = xt_pool.tile([128, NOCT * 128], F16, tag="xt", name="xt_h")
    nc.sync.dma_start(xt_v[:, :4096], aps["xv"][:, :4096])
    nc.scalar.dma_start(w_p_sb[:], aps["w_p"])
    nc.sync.dma_start(xt_v[:, 4096:], aps["xv"][:, 4096:])
    nc.scalar.dma_start(w_t_sb[:, :4096], aps["w_t"][:, :4096])
    nc.sync.dma_start(a16_sb[:], aps["a16"])
    nc.scalar.dma_start(w_cba_sb[:, :4096], aps["w_cba"][:, :4096])
    nc.sync.dma_start(w_t_sb[:, 4096:], aps["w_t"][:, 4096:])
    nc.sync.dma_start(w_cba_sb[:, 4096:], aps["w_cba"][:, 4096:])
    nc.sync.dma_start(wpt_sb[:], aps["wpt"])
    nc.sync.dma_start(xt_h[:, :4096], aps["xh"][:, :4096])
    nc.sync.dma_start(xt_h[:, 4096:], aps["xh"][:, 4096:])
    consts = (w_t_sb[:], w_p_sb[:], w_cba_sb[:], a16_sb[:], wpt_sb[:], ident[:])

    st = {}
    for tag in ("v", "h"):
        g_all = g_pool.tile([128, 8 * 512], F16, tag="g", name=f"g_{tag}")
        s_all = s_pool.tile([128, NG * 128], F16, tag="s", name=f"s_{tag}")
        y_sb = y_pool.tile([128, NOCT * 128], F16, tag="y", name=f"y_{tag}")
        st[tag] = {"g_all": g_all, "s_all": s_all, "y_sb": y_sb}
    pools = pools + (st,)

    # PE warmup with real matmuls while the first loads land
    ps_w = psG.tile([128, 512], F32, tag="ps_g", name="warm")
    for r in range(20):
        for j in range(4):
            nc.tensor.matmul(ps_w[:, j * 128:(j + 1) * 128], ident[:], ident[:],
                             start=True, stop=True, skip_group_check=True)

    for q in range(8):
        _phase_G(tc, pools, consts, xt_v, "v", q)
    _recurrence(tc, pools, consts, "v")
    for og in range(16):
        _phase_B(tc, pools, consts, xt_v, "v", og)
    _phase_proj(tc, pools, consts, aps["zv"], "v")
    for q in range(8):
        _phase_G(tc, pools, consts, xt_h, "h", q)
    _recurrence(tc, pools, consts, "h")
    for og in range(16):
        _phase_B(tc, pools, consts, xt_h, "h", og)
    _phase_proj(tc, pools, consts, aps["zh"], "h")
